# revision 4
# baseline (speedup 1.0000x reference)
"""Sharded kNN retrieval kernel for Trainium2 (8 NeuronCores).

Reference computation (nn_ATNLPmodel):
  qn = q / (||q|| + eps);  dn = d / (||d|| + eps)   (rows flattened to D=1024)
  sim = qn @ dn.T                       [B=16, N=400000]
  -> unit_sim = max_n sim, avg_sim = mean_n sim,
     top_cls = db_classes[argmax_n sim], accuracy = mean(top_cls == y)

Strategy: shard the database N-axis across 8 cores (50k rows each), host
pre-transposes each shard to [D, Nc] layout (layout prep only — all math
is on-device). Per core, stream the shard in 100 chunks of 500 rows:
  - sim via f32r matmuls (qT stationary, 8 K-slices of 128 accumulated in
    PSUM)
  - row norms via ACT Square -> ones-matmul (replicated across the 16
    query partitions for free)
  - scale sim by 1/((||q||+eps) * ||d||) in one scalar_tensor_tensor op,
    which also emits the per-chunk row-sum (for avg_sim)
  - per-chunk top-8 values + indices via DVE max/max_index
Host gathers per-chunk partials, rescores the top candidates exactly in
float64 (kills f32r ranking noise), and finishes the tiny combine.
"""
import os
import numpy as np

B = 16
D = 1024
N = 400000
NCORES = 8
NSHARD = N // NCORES          # 50000
CH = 500                      # db rows per chunk
NCHUNK = NSHARD // CH         # 100
S = D // 128                  # 8 contraction slices
EPS = 1e-8
RESCORE_K = 128               # host-rescored candidates per query

LAST_EXEC_NS = None
_CACHE = {}


def _chunk_schedule(iters):
    # repeat the chunk loop `iters` times (identical work; bench-only —
    # kernel() always uses iters=1)
    out = []
    for _ in range(iters):
        out.extend(range(NCHUNK))
    return out


def _build(iters=1):
    import concourse.bacc as bacc
    import concourse.mybir as mybir
    import concourse.tile as tile

    F32 = mybir.dt.float32
    F32R = mybir.dt.float32r
    U32 = mybir.dt.uint32
    AF = mybir.ActivationFunctionType
    OP = mybir.AluOpType

    nc = bacc.Bacc("TRN2", target_bir_lowering=False, debug=False,
                   num_devices=NCORES)
    qp = nc.declare_dram_parameter("q", [B, D], F32, isOutput=False)
    qTp = nc.declare_dram_parameter("qT", [S, 128, B], F32R, isOutput=False)
    DTp = nc.declare_dram_parameter("DT", [S, 128, NSHARD], F32R, isOutput=False)
    onesp = nc.declare_dram_parameter("ones", [128, B], F32R, isOutput=False)
    cmax_o = nc.declare_dram_parameter("cmax", [B, 8 * NCHUNK], F32, isOutput=True)
    cidx_o = nc.declare_dram_parameter("cidx", [B, 8 * NCHUNK], U32, isOutput=True)
    csum_o = nc.declare_dram_parameter("csum", [B, NCHUNK], F32, isOutput=True)

    DTv = DTp.rearrange("s p n -> p s n")

    with tile.TileContext(nc) as tc:
        with (
            tc.tile_pool(name="const", bufs=1) as const,
            tc.tile_pool(name="dt", bufs=3) as dtp,
            tc.tile_pool(name="sq", bufs=2) as sqp,
            tc.tile_pool(name="small", bufs=3) as small,
            tc.tile_pool(name="psum", bufs=2, space="PSUM") as psum,
        ):
            # ---- prologue: query norm scalars ----
            q_t = const.tile([B, D], F32)
            nc.sync.dma_start(out=q_t[:], in_=qp[:])
            qT_t = const.tile([128, S, B], F32R)
            nc.sync.dma_start(out=qT_t[:], in_=qTp.rearrange("s p b -> p s b"))
            ones_t = const.tile([128, B], F32R)
            nc.sync.dma_start(out=ones_t[:], in_=onesp[:])

            qsq = const.tile([B, D], F32)
            qss = const.tile([B, 1], F32)
            nc.scalar.activation(qsq[:], q_t[:], AF.Square, accum_out=qss[:])
            qnorm = const.tile([B, 1], F32)
            nc.scalar.activation(qnorm[:], qss[:], AF.Sqrt)
            qnormpe = const.tile([B, 1], F32)
            nc.vector.tensor_scalar_add(qnormpe[:], qnorm[:], EPS)
            qrecip = const.tile([B, 1], F32)
            nc.vector.reciprocal(qrecip[:], qnormpe[:])

            cmax_t = const.tile([B, 8 * NCHUNK], F32)
            cidx_t = const.tile([B, 8 * NCHUNK], U32)
            csum_t = const.tile([B, NCHUNK], F32)

            # ---- chunk loop ----
            for j in _chunk_schedule(iters):
                dt_t = dtp.tile([128, S, CH], F32R, tag="dt")
                nc.sync.dma_start(out=dt_t[:], in_=DTv[:, :, j * CH:(j + 1) * CH])
                sq_t = sqp.tile([128, S, CH], F32R, tag="sq")
                nc.scalar.activation(sq_t[:], dt_t[:], AF.Square)

                psim = psum.tile([B, CH], F32, tag="psim")
                pnorm = psum.tile([B, CH], F32, tag="pnorm")
                for s in range(S):
                    nc.tensor.matmul(psim[:], qT_t[:, s, :], dt_t[:, s, :],
                                     start=(s == 0), stop=(s == S - 1))
                for s in range(S):
                    nc.tensor.matmul(pnorm[:], ones_t[:], sq_t[:, s, :],
                                     start=(s == 0), stop=(s == S - 1))

                snorm = small.tile([B, CH], F32, tag="snorm")
                nc.scalar.activation(snorm[:], pnorm[:], AF.Sqrt)
                denom = small.tile([B, CH], F32, tag="denom")
                nc.vector.reciprocal(denom[:], snorm[:])
                sims = small.tile([B, CH], F32, tag="sims")
                nc.vector.scalar_tensor_tensor(
                    out=sims[:], in0=psim[:], scalar=qrecip[:], in1=denom[:],
                    op0=OP.mult, op1=OP.mult,
                    accum_out=csum_t[:, j:j + 1],
                )
                nc.vector.max(cmax_t[:, 8 * j:8 * j + 8], sims[:])
                nc.vector.max_index(cidx_t[:, 8 * j:8 * j + 8],
                                    cmax_t[:, 8 * j:8 * j + 8], sims[:])

            nc.sync.dma_start(out=cmax_o[:], in_=cmax_t[:])
            nc.sync.dma_start(out=cidx_o[:], in_=cidx_t[:])
            nc.sync.dma_start(out=csum_o[:], in_=csum_t[:])
    nc.compile()
    return nc


def kernel(queries, database, db_classes, y):
    global LAST_EXEC_NS
    from concourse.bass_utils import run_bass_kernel_spmd

    q = np.ascontiguousarray(np.asarray(queries), dtype=np.float32).reshape(B, D)
    db = np.ascontiguousarray(np.asarray(database), dtype=np.float32).reshape(N, D)
    db_classes = np.asarray(db_classes)
    y = np.asarray(y)

    if "nc" not in _CACHE:
        _CACHE["nc"] = _build()
    nc = _CACHE["nc"]

    qT = np.ascontiguousarray(q.T).reshape(S, 128, B)
    ones = np.ones((128, B), np.float32)
    in_maps = []
    for c in range(NCORES):
        shard = db[c * NSHARD:(c + 1) * NSHARD]
        DT = np.ascontiguousarray(shard.T).reshape(S, 128, NSHARD)
        in_maps.append({"q": q, "qT": qT, "DT": DT, "ones": ones})

    trace = bool(os.environ.get("KNN_TRACE"))
    res = run_bass_kernel_spmd(nc, in_maps, list(range(NCORES)), trace=trace)
    LAST_EXEC_NS = res.exec_time_ns

    # ---- host combine (gather/unshard glue) ----
    # candidate global index for every (core, chunk, slot)
    vals = np.stack([res.results[c]["cmax"] for c in range(NCORES)])   # [8,B,800]
    idxs = np.stack([res.results[c]["cidx"] for c in range(NCORES)])   # [8,B,800]
    sums = np.stack([res.results[c]["csum"] for c in range(NCORES)])   # [8,B,100]

    chunk_base = (np.arange(NCORES)[:, None] * NSHARD
                  + np.arange(NCHUNK)[None, :] * CH)                   # [8,100]
    gidx = (idxs.reshape(NCORES, B, NCHUNK, 8)
            + chunk_base[:, None, :, None]).astype(np.int64)           # [8,B,100,8]

    vals_q = vals.reshape(NCORES, B, NCHUNK * 8).transpose(1, 0, 2).reshape(B, -1)
    gidx_q = gidx.reshape(NCORES, B, NCHUNK * 8).transpose(1, 0, 2).reshape(B, -1)

    # exact rescore of the top candidates (removes f32r ranking noise)
    qn64 = q.astype(np.float64)
    qn64 /= (np.linalg.norm(qn64, axis=1, keepdims=True) + EPS)
    unit_sim = np.empty(B, np.float32)
    best = np.empty(B, np.int64)
    k = min(RESCORE_K, vals_q.shape[1])
    for b in range(B):
        cand = np.unique(gidx_q[b][np.argpartition(-vals_q[b], k - 1)[:k]])
        rows = db[cand].astype(np.float64)
        rsim = (rows @ qn64[b]) / (np.linalg.norm(rows, axis=1) + EPS)
        top = np.argmax(rsim)
        # first-occurrence tie-break on the global index like jnp.argmax
        tmax = rsim.max()
        ties = cand[rsim >= tmax - 0.0]
        best[b] = ties.min() if len(ties) > 1 else cand[top]
        unit_sim[b] = np.float32(rsim[top])

    avg_sim = (sums.astype(np.float64).sum(axis=(0, 2)) / N).astype(np.float32)
    top_cls = db_classes[best].astype(np.int32)
    accuracy = np.float32((top_cls == y).mean())
    return unit_sim, avg_sim, top_cls, accuracy


# revision 7
# speedup vs baseline: 1.0346x; 1.0346x over previous
"""Sharded kNN retrieval kernel for Trainium2 (8 NeuronCores).

Reference computation (nn_ATNLPmodel):
  qn = q / (||q|| + eps);  dn = d / (||d|| + eps)   (rows flattened to D=1024)
  sim = qn @ dn.T                       [B=16, N=400000]
  -> unit_sim = max_n sim, avg_sim = mean_n sim,
     top_cls = db_classes[argmax_n sim], accuracy = mean(top_cls == y)

Strategy: shard the database N-axis across 8 cores (50k rows each), host
pre-transposes each shard to [D, Nc] layout (layout prep only — all math
is on-device). Per core, stream the shard in 100 chunks of 500 rows:
  - sim via f32r matmuls (qT stationary, 8 K-slices of 128 accumulated in
    PSUM)
  - row norms via ACT Square -> ones-matmul (replicated across the 16
    query partitions for free)
  - scale sim by 1/((||q||+eps) * ||d||) in one scalar_tensor_tensor op,
    which also emits the per-chunk row-sum (for avg_sim)
  - per-chunk top-8 values + indices via DVE max/max_index
Host gathers per-chunk partials, rescores the top candidates exactly in
float64 (kills f32r ranking noise), and finishes the tiny combine.
"""
import os
import numpy as np

B = 16
D = 1024
N = 400000
NCORES = 8
NSHARD = N // NCORES          # 50000
CH = 1024                     # db rows per full chunk
NCHUNK = -(-NSHARD // CH)     # 49 (48 full + one 768 tail)
S = D // 128                  # 8 contraction slices
EPS = 1e-8
RESCORE_K = 128               # host-rescored candidates per query

LAST_EXEC_NS = None
_CACHE = {}


def _chunks():
    # (chunk_index, start_row, width)
    out = []
    for j in range(NCHUNK):
        start = j * CH
        out.append((j, start, min(CH, NSHARD - start)))
    return out


def _chunk_schedule(iters):
    # repeat the chunk loop `iters` times (identical work; bench-only —
    # kernel() always uses iters=1)
    out = []
    for _ in range(iters):
        out.extend(_chunks())
    return out


def _build(iters=1):
    import concourse.bacc as bacc
    import concourse.mybir as mybir
    import concourse.tile as tile

    F32 = mybir.dt.float32
    F32R = mybir.dt.float32r
    U32 = mybir.dt.uint32
    AF = mybir.ActivationFunctionType
    OP = mybir.AluOpType

    nc = bacc.Bacc("TRN2", target_bir_lowering=False, debug=False,
                   num_devices=NCORES)
    qp = nc.declare_dram_parameter("q", [B, D], F32, isOutput=False)
    qTp = nc.declare_dram_parameter("qT", [S, 128, B], F32R, isOutput=False)
    DTp = nc.declare_dram_parameter("DT", [S, 128, NSHARD], F32R, isOutput=False)
    onesp = nc.declare_dram_parameter("ones", [128, B], F32R, isOutput=False)
    cmax_o = nc.declare_dram_parameter("cmax", [B, 8 * NCHUNK], F32, isOutput=True)
    cidx_o = nc.declare_dram_parameter("cidx", [B, 8 * NCHUNK], U32, isOutput=True)
    csum_o = nc.declare_dram_parameter("csum", [B, NCHUNK], F32, isOutput=True)

    DTv = DTp.rearrange("s p n -> p s n")

    with tile.TileContext(nc) as tc:
        with (
            tc.tile_pool(name="const", bufs=1) as const,
            tc.tile_pool(name="dt", bufs=3) as dtp,
            tc.tile_pool(name="sq", bufs=2) as sqp,
            tc.tile_pool(name="small", bufs=2) as small,
            tc.tile_pool(name="psum", bufs=2, space="PSUM") as psum,
        ):
            # ---- prologue: query norm scalars ----
            q_t = const.tile([B, D], F32)
            nc.sync.dma_start(out=q_t[:], in_=qp[:])
            qT_t = const.tile([128, S, B], F32R)
            nc.sync.dma_start(out=qT_t[:], in_=qTp.rearrange("s p b -> p s b"))
            ones_t = const.tile([128, B], F32R)
            nc.sync.dma_start(out=ones_t[:], in_=onesp[:])

            qsq = const.tile([B, D], F32)
            qss = const.tile([B, 1], F32)
            nc.scalar.activation(qsq[:], q_t[:], AF.Square, accum_out=qss[:])
            qnorm = const.tile([B, 1], F32)
            nc.scalar.activation(qnorm[:], qss[:], AF.Sqrt)
            qnormpe = const.tile([B, 1], F32)
            nc.vector.tensor_scalar_add(qnormpe[:], qnorm[:], EPS)
            qrecip = const.tile([B, 1], F32)
            nc.vector.reciprocal(qrecip[:], qnormpe[:])

            cmax_t = const.tile([B, 8 * NCHUNK], F32)
            cidx_t = const.tile([B, 8 * NCHUNK], U32)
            csum_t = const.tile([B, NCHUNK], F32)

            # ---- chunk loop ----
            for j, start, w in _chunk_schedule(iters):
                dt_t = dtp.tile([128, S, CH], F32R, tag="dt")
                nc.sync.dma_start(out=dt_t[:, :, :w], in_=DTv[:, :, start:start + w])
                sq_t = sqp.tile([128, S, CH], F32R, tag="sq")
                nc.scalar.activation(sq_t[:, :, :w], dt_t[:, :, :w], AF.Square)

                psim = psum.tile([B, CH], F32, tag="psim")
                pnorm = psum.tile([B, CH], F32, tag="pnorm")
                for seg0 in range(0, w, 512):
                    seg = slice(seg0, min(seg0 + 512, w))
                    for s in range(S):
                        nc.tensor.matmul(psim[:, seg], qT_t[:, s, :],
                                         dt_t[:, s, seg],
                                         start=(s == 0), stop=(s == S - 1))
                    for s in range(S):
                        nc.tensor.matmul(pnorm[:, seg], ones_t[:],
                                         sq_t[:, s, seg],
                                         start=(s == 0), stop=(s == S - 1))

                snorm = small.tile([B, CH], F32, tag="snorm")
                nc.scalar.activation(snorm[:, :w], pnorm[:, :w], AF.Sqrt)
                denom = small.tile([B, CH], F32, tag="denom")
                nc.vector.reciprocal(denom[:, :w], snorm[:, :w])
                sims = small.tile([B, CH], F32, tag="sims")
                nc.vector.scalar_tensor_tensor(
                    out=sims[:, :w], in0=psim[:, :w], scalar=qrecip[:],
                    in1=denom[:, :w],
                    op0=OP.mult, op1=OP.mult,
                    accum_out=csum_t[:, j:j + 1],
                )
                nc.vector.max(cmax_t[:, 8 * j:8 * j + 8], sims[:, :w])
                nc.vector.max_index(cidx_t[:, 8 * j:8 * j + 8],
                                    cmax_t[:, 8 * j:8 * j + 8], sims[:, :w])

            nc.sync.dma_start(out=cmax_o[:], in_=cmax_t[:])
            nc.sync.dma_start(out=cidx_o[:], in_=cidx_t[:])
            nc.sync.dma_start(out=csum_o[:], in_=csum_t[:])
    nc.compile()
    return nc


def kernel(queries, database, db_classes, y):
    global LAST_EXEC_NS
    from concourse.bass_utils import run_bass_kernel_spmd

    q = np.ascontiguousarray(np.asarray(queries), dtype=np.float32).reshape(B, D)
    db = np.ascontiguousarray(np.asarray(database), dtype=np.float32).reshape(N, D)
    db_classes = np.asarray(db_classes)
    y = np.asarray(y)

    if "nc" not in _CACHE:
        _CACHE["nc"] = _build()
    nc = _CACHE["nc"]

    qT = np.ascontiguousarray(q.T).reshape(S, 128, B)
    ones = np.ones((128, B), np.float32)
    in_maps = []
    for c in range(NCORES):
        shard = db[c * NSHARD:(c + 1) * NSHARD]
        DT = np.ascontiguousarray(shard.T).reshape(S, 128, NSHARD)
        in_maps.append({"q": q, "qT": qT, "DT": DT, "ones": ones})

    trace = bool(os.environ.get("KNN_TRACE"))
    res = run_bass_kernel_spmd(nc, in_maps, list(range(NCORES)), trace=trace)
    LAST_EXEC_NS = res.exec_time_ns

    # ---- host combine (gather/unshard glue) ----
    # candidate global index for every (core, chunk, slot)
    vals = np.stack([res.results[c]["cmax"] for c in range(NCORES)])   # [8,B,800]
    idxs = np.stack([res.results[c]["cidx"] for c in range(NCORES)])   # [8,B,800]
    sums = np.stack([res.results[c]["csum"] for c in range(NCORES)])   # [8,B,100]

    chunk_base = (np.arange(NCORES)[:, None] * NSHARD
                  + np.arange(NCHUNK)[None, :] * CH)                   # [8,100]
    gidx = (idxs.reshape(NCORES, B, NCHUNK, 8)
            + chunk_base[:, None, :, None]).astype(np.int64)           # [8,B,100,8]

    vals_q = vals.reshape(NCORES, B, NCHUNK * 8).transpose(1, 0, 2).reshape(B, -1)
    gidx_q = gidx.reshape(NCORES, B, NCHUNK * 8).transpose(1, 0, 2).reshape(B, -1)

    # exact rescore of the top candidates (removes f32r ranking noise)
    qn64 = q.astype(np.float64)
    qn64 /= (np.linalg.norm(qn64, axis=1, keepdims=True) + EPS)
    unit_sim = np.empty(B, np.float32)
    best = np.empty(B, np.int64)
    k = min(RESCORE_K, vals_q.shape[1])
    for b in range(B):
        cand = np.unique(gidx_q[b][np.argpartition(-vals_q[b], k - 1)[:k]])
        rows = db[cand].astype(np.float64)
        rsim = (rows @ qn64[b]) / (np.linalg.norm(rows, axis=1) + EPS)
        top = np.argmax(rsim)
        # first-occurrence tie-break on the global index like jnp.argmax
        tmax = rsim.max()
        ties = cand[rsim >= tmax - 0.0]
        best[b] = ties.min() if len(ties) > 1 else cand[top]
        unit_sim[b] = np.float32(rsim[top])

    avg_sim = (sums.astype(np.float64).sum(axis=(0, 2)) / N).astype(np.float32)
    top_cls = db_classes[best].astype(np.int32)
    accuracy = np.float32((top_cls == y).mean())
    return unit_sim, avg_sim, top_cls, accuracy


# revision 11
# speedup vs baseline: 1.1812x; 1.1417x over previous
"""Sharded kNN retrieval kernel for Trainium2 (8 NeuronCores).

Reference computation (nn_ATNLPmodel):
  qn = q / (||q|| + eps);  dn = d / (||d|| + eps)   (rows flattened to D=1024)
  sim = qn @ dn.T                       [B=16, N=400000]
  -> unit_sim = max_n sim, avg_sim = mean_n sim,
     top_cls = db_classes[argmax_n sim], accuracy = mean(top_cls == y)

Strategy: shard the database N-axis across 8 cores (50k rows each), host
pre-transposes each shard to [D, Nc] layout (layout prep only — all math
is on-device). Per core, stream the shard in 100 chunks of 500 rows:
  - sim via f32r matmuls (qT stationary, 8 K-slices of 128 accumulated in
    PSUM)
  - row norms via ACT Square -> ones-matmul (replicated across the 16
    query partitions for free)
  - scale sim by 1/((||q||+eps) * ||d||) in one scalar_tensor_tensor op,
    which also emits the per-chunk row-sum (for avg_sim)
  - per-chunk top-8 values + indices via DVE max/max_index
Host gathers per-chunk partials, rescores the top candidates exactly in
float64 (kills f32r ranking noise), and finishes the tiny combine.
"""
import os
import numpy as np

B = 16
D = 1024
N = 400000
NCORES = 8
NSHARD = N // NCORES          # 50000
CH = 1024                     # db rows per full chunk
NCHUNK = -(-NSHARD // CH)     # 49 (48 full + one 768 tail)
S = D // 128                  # 8 contraction slices
EPS = 1e-8
RESCORE_K = 128               # host-rescored candidates per query

LAST_EXEC_NS = None
_CACHE = {}


def _chunks():
    # (chunk_index, start_row, width)
    out = []
    for j in range(NCHUNK):
        start = j * CH
        out.append((j, start, min(CH, NSHARD - start)))
    return out


def _chunk_schedule(iters):
    # repeat the chunk loop `iters` times (identical work; bench-only —
    # kernel() always uses iters=1)
    out = []
    for _ in range(iters):
        out.extend(_chunks())
    return out


def _build(iters=1):
    import concourse.bacc as bacc
    import concourse.mybir as mybir
    import concourse.tile as tile

    F32 = mybir.dt.float32
    F32R = mybir.dt.float32r
    U32 = mybir.dt.uint32
    AF = mybir.ActivationFunctionType
    OP = mybir.AluOpType

    BF16 = mybir.dt.bfloat16
    nc = bacc.Bacc("TRN2", target_bir_lowering=False, debug=False,
                   num_devices=NCORES)
    qp = nc.declare_dram_parameter("q", [B, D], F32, isOutput=False)
    qTp = nc.declare_dram_parameter("qT", [S, 128, B], F32R, isOutput=False)
    DTp = nc.declare_dram_parameter("DT", [S, 128, NSHARD], F32R, isOutput=False)
    onesp = nc.declare_dram_parameter("ones", [128, B], BF16, isOutput=False)
    cmax_o = nc.declare_dram_parameter("cmax", [B, 8 * NCHUNK], F32, isOutput=True)
    cidx_o = nc.declare_dram_parameter("cidx", [B, 8 * NCHUNK], U32, isOutput=True)
    csum_o = nc.declare_dram_parameter("csum", [B, NCHUNK], F32, isOutput=True)

    DTv = DTp.rearrange("s p n -> p s n")

    with tile.TileContext(nc) as tc:
        with (
            tc.tile_pool(name="const", bufs=1) as const,
            tc.tile_pool(name="dt", bufs=3) as dtp,
            tc.tile_pool(name="sq", bufs=2) as sqp,
            tc.tile_pool(name="small", bufs=2) as small,
            tc.tile_pool(name="psum", bufs=2, space="PSUM") as psum,
        ):
            # ---- prologue: query norm scalars ----
            q_t = const.tile([B, D], F32)
            nc.sync.dma_start(out=q_t[:], in_=qp[:])
            qT_t = const.tile([128, S, B], F32R)
            nc.sync.dma_start(out=qT_t[:], in_=qTp.rearrange("s p b -> p s b"))
            ones_t = const.tile([128, B], BF16)
            nc.sync.dma_start(out=ones_t[:], in_=onesp[:])

            qsq = const.tile([B, D], F32)
            qss = const.tile([B, 1], F32)
            nc.scalar.activation(qsq[:], q_t[:], AF.Square, accum_out=qss[:])
            qnorm = const.tile([B, 1], F32)
            nc.scalar.activation(qnorm[:], qss[:], AF.Sqrt)
            qnormpe = const.tile([B, 1], F32)
            nc.vector.tensor_scalar_add(qnormpe[:], qnorm[:], EPS)
            qrecip = const.tile([B, 1], F32)
            nc.vector.reciprocal(qrecip[:], qnormpe[:])

            cmax_t = const.tile([B, 8 * NCHUNK], F32)
            cidx_t = const.tile([B, 8 * NCHUNK], U32)
            csum_t = const.tile([B, NCHUNK], F32)

            # ---- chunk loop ----
            for j, start, w in _chunk_schedule(iters):
                dt_t = dtp.tile([128, S, CH], F32R, tag="dt")
                nc.sync.dma_start(out=dt_t[:, :, :w], in_=DTv[:, :, start:start + w])
                sq_t = sqp.tile([128, S, CH], BF16, tag="sq")
                nc.scalar.activation(sq_t[:, :, :w], dt_t[:, :, :w], AF.Square)

                # sim (f32r) on PE col-tile T0 -> psum rows 0:16; norms (bf16)
                # on col-tile T1 -> psum rows 32:48. The two streams run
                # concurrently on different column groups of the PE array.
                ps = psum.tile([48, CH], F32, tag="ps")
                for seg0 in range(0, w, 512):
                    seg = slice(seg0, min(seg0 + 512, w))
                    for s in range(S):
                        nc.tensor.matmul(ps[0:B, seg], qT_t[:, s, :],
                                         dt_t[:, s, seg],
                                         start=(s == 0), stop=(s == S - 1),
                                         tile_position=(0, 0))
                        nc.tensor.matmul(ps[32:32 + B, seg], ones_t[:],
                                         sq_t[:, s, seg],
                                         start=(s == 0), stop=(s == S - 1),
                                         tile_position=(0, 32))

                snorm = small.tile([B, CH], F32, tag="snorm")
                nc.scalar.activation(snorm[:, :w], ps[32:32 + B, :w], AF.Sqrt)
                denom = small.tile([B, CH], F32, tag="denom")
                nc.vector.reciprocal(denom[:, :w], snorm[:, :w])
                sims = small.tile([B, CH], F32, tag="sims")
                nc.vector.scalar_tensor_tensor(
                    out=sims[:, :w], in0=ps[0:B, :w], scalar=qrecip[:],
                    in1=denom[:, :w],
                    op0=OP.mult, op1=OP.mult,
                    accum_out=csum_t[:, j:j + 1],
                )
                nc.vector.max(cmax_t[:, 8 * j:8 * j + 8], sims[:, :w])
                nc.vector.max_index(cidx_t[:, 8 * j:8 * j + 8],
                                    cmax_t[:, 8 * j:8 * j + 8], sims[:, :w])

            nc.sync.dma_start(out=cmax_o[:], in_=cmax_t[:])
            nc.sync.dma_start(out=cidx_o[:], in_=cidx_t[:])
            nc.sync.dma_start(out=csum_o[:], in_=csum_t[:])
    nc.compile()
    return nc


def kernel(queries, database, db_classes, y):
    global LAST_EXEC_NS
    from concourse.bass_utils import run_bass_kernel_spmd

    q = np.ascontiguousarray(np.asarray(queries), dtype=np.float32).reshape(B, D)
    db = np.ascontiguousarray(np.asarray(database), dtype=np.float32).reshape(N, D)
    db_classes = np.asarray(db_classes)
    y = np.asarray(y)

    if "nc" not in _CACHE:
        _CACHE["nc"] = _build()
    nc = _CACHE["nc"]

    import ml_dtypes
    qT = np.ascontiguousarray(q.T).reshape(S, 128, B)
    ones = np.ones((128, B), ml_dtypes.bfloat16)
    in_maps = []
    for c in range(NCORES):
        shard = db[c * NSHARD:(c + 1) * NSHARD]
        DT = np.ascontiguousarray(shard.T).reshape(S, 128, NSHARD)
        in_maps.append({"q": q, "qT": qT, "DT": DT, "ones": ones})

    trace = bool(os.environ.get("KNN_TRACE"))
    res = run_bass_kernel_spmd(nc, in_maps, list(range(NCORES)), trace=trace)
    LAST_EXEC_NS = res.exec_time_ns

    # ---- host combine (gather/unshard glue) ----
    # candidate global index for every (core, chunk, slot)
    vals = np.stack([res.results[c]["cmax"] for c in range(NCORES)])   # [8,B,800]
    idxs = np.stack([res.results[c]["cidx"] for c in range(NCORES)])   # [8,B,800]
    sums = np.stack([res.results[c]["csum"] for c in range(NCORES)])   # [8,B,100]

    chunk_base = (np.arange(NCORES)[:, None] * NSHARD
                  + np.arange(NCHUNK)[None, :] * CH)                   # [8,100]
    gidx = (idxs.reshape(NCORES, B, NCHUNK, 8)
            + chunk_base[:, None, :, None]).astype(np.int64)           # [8,B,100,8]

    vals_q = vals.reshape(NCORES, B, NCHUNK * 8).transpose(1, 0, 2).reshape(B, -1)
    gidx_q = gidx.reshape(NCORES, B, NCHUNK * 8).transpose(1, 0, 2).reshape(B, -1)

    # exact rescore of the top candidates (removes f32r ranking noise)
    qn64 = q.astype(np.float64)
    qn64 /= (np.linalg.norm(qn64, axis=1, keepdims=True) + EPS)
    unit_sim = np.empty(B, np.float32)
    best = np.empty(B, np.int64)
    k = min(RESCORE_K, vals_q.shape[1])
    for b in range(B):
        cand = np.unique(gidx_q[b][np.argpartition(-vals_q[b], k - 1)[:k]])
        rows = db[cand].astype(np.float64)
        rsim = (rows @ qn64[b]) / (np.linalg.norm(rows, axis=1) + EPS)
        top = np.argmax(rsim)
        # first-occurrence tie-break on the global index like jnp.argmax
        tmax = rsim.max()
        ties = cand[rsim >= tmax - 0.0]
        best[b] = ties.min() if len(ties) > 1 else cand[top]
        unit_sim[b] = np.float32(rsim[top])

    avg_sim = (sums.astype(np.float64).sum(axis=(0, 2)) / N).astype(np.float32)
    top_cls = db_classes[best].astype(np.int32)
    accuracy = np.float32((top_cls == y).mean())
    return unit_sim, avg_sim, top_cls, accuracy
